# revision 39
# baseline (speedup 1.0000x reference)
"""GQA attention layer (dense transformer block) on 8 TRN2 NeuronCores.

Tensor-parallel sharding over heads: each core owns 4 q-heads + 1 kv-head
(wq/wk/wv column shards, wo row shard), computes a partial output
[2048, 2048] in bf16, and the host sums the 8 partials (the row-parallel
all-reduce) to produce the full f32 output.

Per-core dataflow (all activations kept transposed, [feature, seq]; all
matmul operands bf16 with fp32 PSUM accumulation):
  xT preloaded to SBUF once (no per-tile DMA waits in stage A)
  qT = wq_c.T @ xT         kvT = wkv_c.T @ xT          (PE)
  RoPE via a [128,128] +-1 rotation matmul + DVE combine with cos/sin
  stage B is one continuous stream over 64 key blocks: head PAIRS run
  concurrently on PE row-groups 0-63/64-127, pair1 trails pair0 by one
  block so each tick issues one exact exp (ACT) and one Schraudolph
  fast-exp (DVE int16-write, bf16-bitcast) on opposite kb parities;
  q-span transitions are rolling per-pair (no global barrier).
  [rowsum; oT_h] = [1|v].T @ E    (PE accumulate over key chunks)
  normalization off the PSUM path: one-copy evacuation, approx-recip,
  1/rowsum broadcast via a tiny K=1 PE matmul, DVE multiply.
  out_partial = oT.T @ wo_c       (PE, natural layout out, bf16 store)
"""
import sys

sys.path.insert(0, "/opt/trn_rl_repo")

import numpy as np
import ml_dtypes
import concourse.bass as bass
import concourse.mybir as mybir
import concourse.tile as tile
from concourse import bacc
from concourse.bass_utils import run_bass_kernel_spmd

F32 = mybir.dt.float32
BF16 = mybir.dt.bfloat16
AF = mybir.ActivationFunctionType
NPBF16 = np.dtype(ml_dtypes.bfloat16)

S = 2048          # sequence length
D = 2048          # model dim
HD = 64           # head dim
HLOC = 4          # q heads per core
NCORES = 8
QW = HLOC * HD    # 256, local q width
KC = S // 128     # 16 key chunks
NS = 4            # x / q-span slices of 512
ROPE_BASE = 10000.0
SCALE = 0.125     # 1/sqrt(HD), applied inside exp
# Schraudolph fast-exp constants (bf16 bit pattern via int16 write):
#   i16 = round(score * SCALE * 128/ln2 + (127*128 - 5.5)); bitcast -> bf16
C0S = SCALE * 128.0 / float(np.log(2.0))
C1S = 127.0 * 128.0 - 5.5 - 1.875   # -1.875 cancels the +1% mean bias
INT16 = mybir.dt.int16


def _build_program():
    nc = bacc.Bacc(None, target_bir_lowering=False)

    xt = nc.dram_tensor("xt", [D, S], BF16, kind="ExternalInput")
    wq_d = nc.dram_tensor("wq_p", [128, KC, QW], BF16, kind="ExternalInput")
    wkv_d = nc.dram_tensor("wkv_p", [128, KC, 128], BF16, kind="ExternalInput")
    wo_d = nc.dram_tensor("wo_p", [128, 2, D], BF16, kind="ExternalInput")
    cos_d = nc.dram_tensor("cos2", [128, S], BF16, kind="ExternalInput")
    sin_d = nc.dram_tensor("sin2", [128, S], BF16, kind="ExternalInput")
    rotq_d = nc.dram_tensor("rot_q", [128, 128], BF16, kind="ExternalInput")
    rotk_d = nc.dram_tensor("rot_k", [128, 64], BF16, kind="ExternalInput")
    id64_d = nc.dram_tensor("id64", [128, 64], BF16, kind="ExternalInput")
    ones_d = nc.dram_tensor("ones_col", [128, KC], BF16, kind="ExternalInput")
    bco_d = nc.dram_tensor("bc_ones", [1, 64], BF16, kind="ExternalInput")
    out_d = nc.dram_tensor("out", [S, D], BF16, kind="ExternalOutput")

    with tile.TileContext(nc) as tc:
        with (
            tc.tile_pool(name="consts", bufs=1) as consts,
            tc.tile_pool(name="big", bufs=1) as big,
        ):
            # wq/wkv on the fast HW DGE FIRST (stage A's first matmul blocks on
            # them), early kc chunks before the rest; xT loaded COLUMN-major
            # (all kc for the first two n-spans, then the rest) so stage A's
            # n=0/n=1 chains never wait on DMA.
            wq_sb = consts.tile([128, KC, QW], BF16)
            wkv_sb = consts.tile([128, KC, 128], BF16)
            xt_sb = big.tile([128, KC, S], BF16)
            for lo, hi in ((0, 4), (4, 8), (8, 12), (12, KC)):
                nc.sync.dma_start(wq_sb[:, lo:hi, :], wq_d[:, lo:hi, :])
                nc.sync.dma_start(wkv_sb[:, lo:hi, :], wkv_d[:, lo:hi, :])
                for kc in range(lo, hi):
                    nc.sync.dma_start(xt_sb[:, kc, 0:512], xt[kc * 128:(kc + 1) * 128, 0:512])
            for kc in range(KC):
                nc.sync.dma_start(xt_sb[:, kc, 512:1024], xt[kc * 128:(kc + 1) * 128, 512:1024])
            for kc in range(KC):
                nc.sync.dma_start(xt_sb[:, kc, 1024:2048], xt[kc * 128:(kc + 1) * 128, 1024:2048])
            rotq_sb = consts.tile([128, 128], BF16)
            nc.gpsimd.dma_start(rotq_sb[:], rotq_d[:, :])
            rotk_sb = consts.tile([128, 64], BF16)
            nc.gpsimd.dma_start(rotk_sb[:], rotk_d[:, :])
            id64_sb = consts.tile([128, 64], BF16)
            nc.gpsimd.dma_start(id64_sb[:], id64_d[:, :])
            cos_sb = consts.tile([128, S], BF16)
            nc.gpsimd.dma_start(cos_sb[:], cos_d[:, :])
            sin_sb = consts.tile([128, S], BF16)
            nc.gpsimd.dma_start(sin_sb[:], sin_d[:, :])
            wo_sb = consts.tile([128, 2, D], BF16)
            nc.gpsimd.dma_start(wo_sb[:], wo_d[:, :, :])

            # persistent activations
            qTr = [big.tile([128, S], BF16, name=f"qTr{j}", tag=f"qTr{j}") for j in range(2)]
            kTr = big.tile([128, S], BF16)  # k-rope duplicated in both halves
            kvT = big.tile([128, S], BF16)
            # ones in column 0 so the av matmul puts the rowsum on PSUM
            # partition 0 (reciprocal_approx_fast misreads non-zero base
            # partitions); v in columns 64-127 so the value rows sit on
            # partition base 64 (64-partition engine APs require base 0/64).
            # Columns 1-63 are never read downstream.
            v_aug = big.tile([128, KC, 128], BF16)
            nc.gpsimd.dma_start(v_aug[:, :, 0:1], ones_d.ap().rearrange("p (c o) -> p c o", o=1))
            bco_sb = consts.tile([1, 64], BF16)
            nc.gpsimd.dma_start(bco_sb[:], bco_d[:, :])
            oT = [big.tile([128, S], BF16, name=f"oT{j}", tag=f"oT{j}") for j in range(2)]

            # ---------------- stage A: projections + rope + v transpose
            with (
                tc.tile_pool(name="psA", bufs=1, space="PSUM") as psA,
                tc.tile_pool(name="tmpA", bufs=3) as tmpA,
            ):
                for n in range(NS):
                    nsl = bass.ts(n, 512)
                    q0_ps = psA.tile([128, 512], F32, tag="q0", bufs=2)
                    q1_ps = psA.tile([128, 512], F32, tag="q1", bufs=2)
                    kv_ps = psA.tile([128, 512], F32, tag="kv", bufs=2)
                    for kc in range(KC):
                        st_ = kc == 0
                        sp_ = kc == KC - 1
                        xsl = xt_sb[:, kc, nsl]
                        nc.tensor.matmul(q0_ps[:], wq_sb[:, kc, 0:128], xsl, start=st_, stop=sp_)
                        nc.tensor.matmul(q1_ps[:], wq_sb[:, kc, 128:256], xsl, start=st_, stop=sp_)
                        nc.tensor.matmul(kv_ps[:], wkv_sb[:, kc, :], xsl, start=st_, stop=sp_)

                    # rope for the two q tiles
                    for jb, ps in ((0, q0_ps), (1, q1_ps)):
                        q_sb = tmpA.tile([128, 512], BF16, tag=f"q{jb}sb")
                        nc.scalar.copy(q_sb[:], ps[:])
                        rot_ps = psA.tile([128, 512], F32, tag="rot", bufs=1)
                        nc.tensor.matmul(rot_ps[:], rotq_sb[:], q_sb[:], start=True, stop=True)
                        t_cos = tmpA.tile([128, 512], BF16, tag="tc", bufs=2)
                        nc.vector.tensor_mul(t_cos[:], q_sb[:], cos_sb[:, nsl])
                        t_sin = tmpA.tile([128, 512], BF16, tag="tsn", bufs=2)
                        nc.vector.tensor_mul(t_sin[:], rot_ps[:], sin_sb[:, nsl])
                        nc.vector.tensor_add(qTr[jb][:, nsl], t_cos[:], t_sin[:])

                    # kv: copy, k-rope, v transpose
                    nc.scalar.copy(kvT[:, nsl], kv_ps[:])
                    rk_ps = psA.tile([128, 512], F32, tag="rot", bufs=1)
                    nc.tensor.matmul(rk_ps[0:64, :], rotk_sb[:], kvT[:, nsl], start=True, stop=True)
                    tk_cos = tmpA.tile([128, 512], BF16, tag="tc", bufs=2)
                    nc.vector.tensor_mul(tk_cos[0:64, :], kvT[0:64, nsl], cos_sb[0:64, nsl])
                    tk_sin = tmpA.tile([128, 512], BF16, tag="tsn", bufs=2)
                    nc.vector.tensor_mul(tk_sin[0:64, :], rk_ps[0:64, :], sin_sb[0:64, nsl])
                    nc.vector.tensor_add(kTr[0:64, nsl], tk_cos[0:64, :], tk_sin[0:64, :])
                    nc.vector.tensor_add(kTr[64:128, nsl], tk_cos[0:64, :], tk_sin[0:64, :])

                    for j in range(4):
                        ck = 4 * n + j
                        vt_ps = psA.tile([128, 64], BF16, tag="vt", bufs=1)
                        nc.tensor.transpose(
                            vt_ps[:],
                            kvT[64:128, ck * 128:(ck + 1) * 128],
                            id64_sb[64:128, :],
                        )
                        nc.scalar.copy(v_aug[:, ck, 64:128], vt_ps[:])

            # ---------------- stage B: attention as ONE continuous stream.
            # Pair p processes global key-block index (t - p): pair1 trails
            # pair0 by one block, so every tick issues one ACT exp granule and
            # one DVE fast-exp granule (opposite kb parity), and q-span
            # transitions happen per-pair (rolling) — no global boundary that
            # would idle the PE and re-throttle its clock.
            with (
                tc.tile_pool(name="psB", bufs=1, space="PSUM") as psB,
                tc.tile_pool(name="tmpB", bufs=2) as tmpB,
            ):
                prev = [None, None]
                ots = {}              # (p, i) -> current psum accumulator
                pending_bc = [[], []]
                pending_mul = [[], []]

                def av(p, pair):
                    qq_, kb, e = pair
                    st_ = kb == 0
                    sp_ = kb == KC - 1
                    nc.tensor.matmul(ots[(p, 0)][:], v_aug[:, kb, :], e[:, 0, :],
                                     start=st_, stop=sp_)
                    nc.tensor.matmul(ots[(p, 1)][:], v_aug[:, kb, :], e[:, 1, :],
                                     start=st_, stop=sp_)

                def do_evacs(p, qq_):
                    # one [128,512] f32 copy per head frees the PSUM bank in a
                    # single ACT op; recip on DVE; bc matmul + mul issued on
                    # later ticks so nothing waits on this chain.
                    recip2 = tmpB.tile([1, 2, 512], F32, tag=f"recip{p}", bufs=2)
                    recip2b = tmpB.tile([1, 2, 512], BF16, tag=f"recipb{p}", bufs=2)
                    for i in range(2):
                        ou = tmpB.tile([128, 512], F32, tag="ou", bufs=6)
                        nc.scalar.copy(ou[:], ots[(p, i)][:, :])
                        nc.vector.reciprocal_approx_fast(recip2[0:1, i, :], ou[0:1, :])
                        pending_bc[p].append((qq_, i, ou, recip2b))
                    nc.scalar.copy(recip2b[:], recip2[:])

                def do_bc(p):
                    while pending_bc[p]:
                        qq_, i, ou, recip2b = pending_bc[p].pop(0)
                        bc = psB.tile([64, 512], F32, name=f"bc{p}{i}",
                                      tag=f"ot{2 * p + i}")
                        nc.tensor.matmul(
                            bc[:], bco_sb[:], recip2b[0:1, i, :],
                            start=True, stop=True,
                        )
                        pending_mul[p].append((qq_, i, ou, bc))

                def flush_mul(p):
                    while pending_mul[p]:
                        qq_, i, ou, bc = pending_mul[p].pop(0)
                        h = 2 * p + i
                        jb, rr = divmod(h, 2)
                        nc.vector.tensor_mul(
                            oT[jb][rr * 64:rr * 64 + 64, bass.ts(qq_, 512)],
                            ou[64:128, :], bc[0:64, :],
                        )

                NB = NS * KC  # 64 global key-block steps per pair
                for t in range(NB + 1):
                    for p in (0, 1):
                        g = t - p
                        if g < 0 or g >= NB:
                            continue
                        qq, kb = divmod(g, KC)
                        qsl = bass.ts(qq, 512)
                        if kb == 0 and prev[p] is not None:
                            # finish the previous span: last av into the OLD
                            # accumulators, then evacuate them in one ACT op
                            # per head (+ recip on DVE).
                            av(p, prev[p])
                            prev[p] = None
                            do_evacs(p, qq - 1)
                        if kb == 1:
                            if qq > 0:
                                do_bc(p)       # PE, recips are done by now
                            if qq > 0:
                                flush_mul(p)   # DVE, before this tick's exp
                            for i in range(2):
                                ots[(p, i)] = psB.tile(
                                    [128, 512], F32,
                                    name=f"ot{2 * p + i}", tag=f"ot{2 * p + i}",
                                    bufs=1,
                                )
                        st_ps = psB.tile([128, 2, 512], F32, tag="st", bufs=2)
                        nc.tensor.matmul(
                            st_ps[:, 0, :],
                            kTr[0:64, kb * 128:(kb + 1) * 128],
                            qTr[p][0:64, qsl], start=True, stop=True,
                        )
                        nc.tensor.matmul(
                            st_ps[:, 1, :],
                            kTr[64:128, kb * 128:(kb + 1) * 128],
                            qTr[p][64:128, qsl], start=True, stop=True,
                        )
                        e_sb = tmpB.tile([128, 2, 512], BF16, tag="e", bufs=6)
                        on_act = (kb % 2 == 0 and kb != 14) or kb == 15
                        if on_act:
                            nc.scalar.activation(e_sb[:], st_ps[:], AF.Exp,
                                                 scale=SCALE)
                        else:
                            nc.vector.tensor_scalar(
                                e_sb.bitcast(INT16)[:], st_ps[:],
                                C0S, C1S,
                                mybir.AluOpType.mult, mybir.AluOpType.add,
                            )
                        if prev[p] is not None:
                            av(p, prev[p])
                        prev[p] = (qq, kb, e_sb)
                for p in (0, 1):
                    av(p, prev[p])
                    do_evacs(p, NS - 1)
                    do_bc(p)
                    flush_mul(p)

            # ---------------- stage C tail: out = oT.T @ wo, bf16 store.
            # One batched row-DMA per srow (4 copies land in one ob tile) —
            # per-DMA post cost on the Sync engine is ~0.6us, so 16 posts
            # instead of 64 matters.
            with (
                tc.tile_pool(name="psC", bufs=1, space="PSUM") as psC,
                tc.tile_pool(name="outp", bufs=3) as outp,
            ):
                for srow in range(S // 128):
                    ob = outp.tile([128, D], BF16, tag="ob")
                    for nn in range(NS):
                        o_ps = psC.tile([128, 512], F32, tag="oc", bufs=4)
                        nc.tensor.matmul(
                            o_ps[:], oT[0][:, srow * 128:(srow + 1) * 128],
                            wo_sb[:, 0, bass.ts(nn, 512)], start=True, stop=False,
                        )
                        nc.tensor.matmul(
                            o_ps[:], oT[1][:, srow * 128:(srow + 1) * 128],
                            wo_sb[:, 1, bass.ts(nn, 512)], start=False, stop=True,
                        )
                        if nn % 2 == 0:
                            nc.vector.tensor_copy(ob[:, bass.ts(nn, 512)], o_ps[:])
                        else:
                            nc.scalar.copy(ob[:, bass.ts(nn, 512)], o_ps[:])
                    nc.sync.dma_start(
                        out_d[srow * 128:(srow + 1) * 128, :], ob[:]
                    )
    nc.compile()
    return nc


_NC_CACHE = None


def _get_program():
    global _NC_CACHE
    if _NC_CACHE is None:
        _NC_CACHE = _build_program()
    return _NC_CACHE


def _host_constants():
    inv_freq = 1.0 / (ROPE_BASE ** (np.arange(0, HD, 2, dtype=np.float32) / HD))
    t = np.arange(S, dtype=np.float32)
    freqs = np.outer(t, inv_freq)
    emb = np.concatenate([freqs, freqs], -1)          # [s, 64]
    cosT = np.cos(emb).T.astype(np.float32)           # [64, s]
    sinT = np.sin(emb).T.astype(np.float32)
    cos2 = np.ascontiguousarray(np.concatenate([cosT, cosT], 0)).astype(NPBF16)
    sin2 = np.ascontiguousarray(np.concatenate([sinT, sinT], 0)).astype(NPBF16)

    R = np.zeros((HD, HD), np.float32)
    for i in range(32):
        R[i, i + 32] = -1.0
        R[i + 32, i] = 1.0
    RT = R.T
    rot_q = np.zeros((128, 128), np.float32)
    rot_q[0:64, 0:64] = RT
    rot_q[64:128, 64:128] = RT
    rot_k = np.zeros((128, 64), np.float32)
    rot_k[0:64, 0:64] = RT
    id64 = np.zeros((128, 64), np.float32)
    id64[64:128, :] = np.eye(64, dtype=np.float32)
    ones_col = np.ones((128, KC), np.float32)
    return (cos2, sin2, rot_q.astype(NPBF16), rot_k.astype(NPBF16),
            id64.astype(NPBF16), ones_col.astype(NPBF16))


def _in_maps(x, wq, wk, wv, wo):
    xT = np.ascontiguousarray(x.reshape(S, D).T.astype(NPBF16))
    cos2, sin2, rot_q, rot_k, id64, ones_col = _host_constants()
    maps = []
    for c in range(NCORES):
        wq_c = wq[:, c * QW:(c + 1) * QW].astype(NPBF16)
        wq_p = np.ascontiguousarray(wq_c.reshape(KC, 128, QW).transpose(1, 0, 2))
        wkv_c = np.concatenate(
            [wk[:, c * HD:(c + 1) * HD], wv[:, c * HD:(c + 1) * HD]], 1
        ).astype(NPBF16)
        wkv_p = np.ascontiguousarray(wkv_c.reshape(KC, 128, 128).transpose(1, 0, 2))
        wo_c = wo[c * QW:(c + 1) * QW, :].astype(NPBF16)
        wo_p = np.ascontiguousarray(wo_c.reshape(2, 128, D).transpose(1, 0, 2))
        maps.append({
            "xt": xT, "wq_p": wq_p, "wkv_p": wkv_p, "wo_p": wo_p,
            "cos2": cos2, "sin2": sin2, "rot_q": rot_q, "rot_k": rot_k,
            "id64": id64, "ones_col": ones_col,
            "bc_ones": np.ones((1, 64), np.float32).astype(NPBF16),
        })
    return maps


def _run(in_maps, trace=False):
    nc = _get_program()
    return run_bass_kernel_spmd(nc, in_maps, core_ids=list(range(NCORES)), trace=trace)


def _gather(res):
    acc = res.results[0]["out"].astype(np.float64)
    for c in range(1, NCORES):
        acc += res.results[c]["out"].astype(np.float64)
    return acc.astype(np.float32).reshape(1, S, D)


def kernel(x, wq, wk, wv, wo):
    x, wq, wk, wv, wo = (np.asarray(a, dtype=np.float32) for a in (x, wq, wk, wv, wo))
    res = _run(_in_maps(x, wq, wk, wv, wo), trace=False)
    return _gather(res)


def run_traced(x, wq, wk, wv, wo):
    """Like kernel() but with NTFF profiling; returns (out, BassKernelResults)."""
    x, wq, wk, wv, wo = (np.asarray(a, dtype=np.float32) for a in (x, wq, wk, wv, wo))
    res = _run(_in_maps(x, wq, wk, wv, wo), trace=True)
    return _gather(res), res


# revision 40
# speedup vs baseline: 1.1885x; 1.1885x over previous
"""GQA attention layer (dense transformer block) on 8 TRN2 NeuronCores.

Tensor-parallel sharding over heads: each core owns 4 q-heads + 1 kv-head
(wq/wk/wv column shards, wo row shard), computes a partial output
[2048, 2048] in bf16, and the host sums the 8 partials (the row-parallel
all-reduce) to produce the full f32 output.

Per-core dataflow (all activations kept transposed, [feature, seq]; all
matmul operands bf16 with fp32 PSUM accumulation):
  xT preloaded to SBUF once (no per-tile DMA waits in stage A)
  qT = wq_c.T @ xT         kvT = wkv_c.T @ xT          (PE)
  RoPE via a [128,128] +-1 rotation matmul + DVE combine with cos/sin
  stage B is one continuous stream over 64 key blocks: head PAIRS run
  concurrently on PE row-groups 0-63/64-127, pair1 trails pair0 by one
  block so each tick issues one exact exp (ACT) and one Schraudolph
  fast-exp (DVE int16-write, bf16-bitcast) on opposite kb parities;
  q-span transitions are rolling per-pair (no global barrier).
  [rowsum; oT_h] = [1|v].T @ E    (PE accumulate over key chunks)
  normalization off the PSUM path: one-copy evacuation, approx-recip,
  1/rowsum broadcast via a tiny K=1 PE matmul, DVE multiply.
  out_partial = oT.T @ wo_c       (PE, natural layout out, bf16 store)
"""
import sys

sys.path.insert(0, "/opt/trn_rl_repo")

import numpy as np
import ml_dtypes
import concourse.bass as bass
import concourse.mybir as mybir
import concourse.tile as tile
from concourse import bacc
from concourse.bass_utils import run_bass_kernel_spmd

F32 = mybir.dt.float32
BF16 = mybir.dt.bfloat16
AF = mybir.ActivationFunctionType
NPBF16 = np.dtype(ml_dtypes.bfloat16)

S = 2048          # sequence length
D = 2048          # model dim
HD = 64           # head dim
HLOC = 4          # q heads per core
NCORES = 8
QW = HLOC * HD    # 256, local q width
KC = S // 128     # 16 key chunks
NS = 4            # x / q-span slices of 512
ROPE_BASE = 10000.0
SCALE = 0.125     # 1/sqrt(HD), applied inside exp
# Schraudolph fast-exp constants (bf16 bit pattern via int16 write):
#   i16 = round(score * SCALE * 128/ln2 + (127*128 - 5.5)); bitcast -> bf16
C0S = SCALE * 128.0 / float(np.log(2.0))
C1S = 127.0 * 128.0 - 5.5 - 1.875   # -1.875 cancels the +1% mean bias
INT16 = mybir.dt.int16


def _build_program():
    nc = bacc.Bacc(None, target_bir_lowering=False)

    xt = nc.dram_tensor("xt", [D, S], BF16, kind="ExternalInput")
    wq_d = nc.dram_tensor("wq_p", [128, KC, QW], BF16, kind="ExternalInput")
    wkv_d = nc.dram_tensor("wkv_p", [128, KC, 128], BF16, kind="ExternalInput")
    wo_d = nc.dram_tensor("wo_p", [128, 2, D], BF16, kind="ExternalInput")
    cos_d = nc.dram_tensor("cos2", [128, S], BF16, kind="ExternalInput")
    sin_d = nc.dram_tensor("sin2", [128, S], BF16, kind="ExternalInput")
    rotq_d = nc.dram_tensor("rot_q", [128, 128], BF16, kind="ExternalInput")
    rotk_d = nc.dram_tensor("rot_k", [128, 64], BF16, kind="ExternalInput")
    id64_d = nc.dram_tensor("id64", [128, 64], BF16, kind="ExternalInput")
    ones_d = nc.dram_tensor("ones_col", [128, KC], BF16, kind="ExternalInput")
    bco_d = nc.dram_tensor("bc_ones", [1, 64], BF16, kind="ExternalInput")
    out_d = nc.dram_tensor("out", [S, D], BF16, kind="ExternalOutput")

    with tile.TileContext(nc) as tc:
        with (
            tc.tile_pool(name="consts", bufs=1) as consts,
            tc.tile_pool(name="big", bufs=1) as big,
        ):
            # wq/wkv on the fast HW DGE FIRST (stage A's first matmul blocks on
            # them), early kc chunks before the rest; xT loaded COLUMN-major
            # (all kc for the first two n-spans, then the rest) so stage A's
            # n=0/n=1 chains never wait on DMA.
            wq_sb = consts.tile([128, KC, QW], BF16)
            wkv_sb = consts.tile([128, KC, 128], BF16)
            xt_sb = big.tile([128, KC, S], BF16)
            for lo, hi in ((0, 4), (4, 10), (10, KC)):
                nc.sync.dma_start(wq_sb[:, lo:hi, :], wq_d[:, lo:hi, :])
                nc.sync.dma_start(wkv_sb[:, lo:hi, :], wkv_d[:, lo:hi, :])
                for kc in range(lo, hi):
                    nc.sync.dma_start(xt_sb[:, kc, 0:512], xt[kc * 128:(kc + 1) * 128, 0:512])
            for kc in range(KC):
                nc.sync.dma_start(xt_sb[:, kc, 512:1024], xt[kc * 128:(kc + 1) * 128, 512:1024])
            for kc in range(KC):
                nc.sync.dma_start(xt_sb[:, kc, 1024:2048], xt[kc * 128:(kc + 1) * 128, 1024:2048])
            rotq_sb = consts.tile([128, 128], BF16)
            nc.gpsimd.dma_start(rotq_sb[:], rotq_d[:, :])
            rotk_sb = consts.tile([128, 64], BF16)
            nc.gpsimd.dma_start(rotk_sb[:], rotk_d[:, :])
            id64_sb = consts.tile([128, 64], BF16)
            nc.gpsimd.dma_start(id64_sb[:], id64_d[:, :])
            cos_sb = consts.tile([128, S], BF16)
            nc.gpsimd.dma_start(cos_sb[:], cos_d[:, :])
            sin_sb = consts.tile([128, S], BF16)
            nc.gpsimd.dma_start(sin_sb[:], sin_d[:, :])
            wo_sb = consts.tile([128, 2, D], BF16)
            nc.gpsimd.dma_start(wo_sb[:], wo_d[:, :, :])

            # persistent activations
            qTr = [big.tile([128, S], BF16, name=f"qTr{j}", tag=f"qTr{j}") for j in range(2)]
            kTr = big.tile([128, S], BF16)  # k-rope duplicated in both halves
            kvT = big.tile([128, S], BF16)
            # ones in column 0 so the av matmul puts the rowsum on PSUM
            # partition 0 (reciprocal_approx_fast misreads non-zero base
            # partitions); v in columns 64-127 so the value rows sit on
            # partition base 64 (64-partition engine APs require base 0/64).
            # Columns 1-63 are never read downstream.
            v_aug = big.tile([128, KC, 128], BF16)
            nc.gpsimd.dma_start(v_aug[:, :, 0:1], ones_d.ap().rearrange("p (c o) -> p c o", o=1))
            bco_sb = consts.tile([1, 64], BF16)
            nc.gpsimd.dma_start(bco_sb[:], bco_d[:, :])
            oT = [big.tile([128, S], BF16, name=f"oT{j}", tag=f"oT{j}") for j in range(2)]

            # ---------------- stage A: projections + rope + v transpose
            with (
                tc.tile_pool(name="psA", bufs=1, space="PSUM") as psA,
                tc.tile_pool(name="tmpA", bufs=3) as tmpA,
            ):
                for n in range(NS):
                    nsl = bass.ts(n, 512)
                    q0_ps = psA.tile([128, 512], F32, tag="q0", bufs=2)
                    q1_ps = psA.tile([128, 512], F32, tag="q1", bufs=2)
                    kv_ps = psA.tile([128, 512], F32, tag="kv", bufs=2)
                    for kc in range(KC):
                        st_ = kc == 0
                        sp_ = kc == KC - 1
                        xsl = xt_sb[:, kc, nsl]
                        nc.tensor.matmul(q0_ps[:], wq_sb[:, kc, 0:128], xsl, start=st_, stop=sp_)
                        nc.tensor.matmul(q1_ps[:], wq_sb[:, kc, 128:256], xsl, start=st_, stop=sp_)
                        nc.tensor.matmul(kv_ps[:], wkv_sb[:, kc, :], xsl, start=st_, stop=sp_)

                    # rope for the two q tiles
                    for jb, ps in ((0, q0_ps), (1, q1_ps)):
                        q_sb = tmpA.tile([128, 512], BF16, tag=f"q{jb}sb")
                        nc.scalar.copy(q_sb[:], ps[:])
                        rot_ps = psA.tile([128, 512], F32, tag="rot", bufs=1)
                        nc.tensor.matmul(rot_ps[:], rotq_sb[:], q_sb[:], start=True, stop=True)
                        t_cos = tmpA.tile([128, 512], BF16, tag="tc", bufs=2)
                        nc.vector.tensor_mul(t_cos[:], q_sb[:], cos_sb[:, nsl])
                        t_sin = tmpA.tile([128, 512], BF16, tag="tsn", bufs=2)
                        nc.vector.tensor_mul(t_sin[:], rot_ps[:], sin_sb[:, nsl])
                        nc.vector.tensor_add(qTr[jb][:, nsl], t_cos[:], t_sin[:])

                    # kv: copy, k-rope, v transpose
                    nc.scalar.copy(kvT[:, nsl], kv_ps[:])
                    rk_ps = psA.tile([128, 512], F32, tag="rot", bufs=1)
                    nc.tensor.matmul(rk_ps[0:64, :], rotk_sb[:], kvT[:, nsl], start=True, stop=True)
                    tk_cos = tmpA.tile([128, 512], BF16, tag="tc", bufs=2)
                    nc.vector.tensor_mul(tk_cos[0:64, :], kvT[0:64, nsl], cos_sb[0:64, nsl])
                    tk_sin = tmpA.tile([128, 512], BF16, tag="tsn", bufs=2)
                    nc.vector.tensor_mul(tk_sin[0:64, :], rk_ps[0:64, :], sin_sb[0:64, nsl])
                    nc.vector.tensor_add(kTr[0:64, nsl], tk_cos[0:64, :], tk_sin[0:64, :])
                    nc.vector.tensor_add(kTr[64:128, nsl], tk_cos[0:64, :], tk_sin[0:64, :])

                    for j in range(4):
                        ck = 4 * n + j
                        vt_ps = psA.tile([128, 64], BF16, tag="vt", bufs=1)
                        nc.tensor.transpose(
                            vt_ps[:],
                            kvT[64:128, ck * 128:(ck + 1) * 128],
                            id64_sb[64:128, :],
                        )
                        nc.scalar.copy(v_aug[:, ck, 64:128], vt_ps[:])

            # ---------------- stage B: attention as ONE continuous stream.
            # Pair p processes global key-block index (t - p): pair1 trails
            # pair0 by one block, so every tick issues one ACT exp granule and
            # one DVE fast-exp granule (opposite kb parity), and q-span
            # transitions happen per-pair (rolling) — no global boundary that
            # would idle the PE and re-throttle its clock.
            with (
                tc.tile_pool(name="psB", bufs=1, space="PSUM") as psB,
                tc.tile_pool(name="tmpB", bufs=2) as tmpB,
            ):
                prev = [None, None]
                ots = {}              # (p, i) -> current psum accumulator
                pending_bc = [[], []]
                pending_mul = [[], []]

                def av(p, pair):
                    qq_, kb, e = pair
                    st_ = kb == 0
                    sp_ = kb == KC - 1
                    nc.tensor.matmul(ots[(p, 0)][:], v_aug[:, kb, :], e[:, 0, :],
                                     start=st_, stop=sp_)
                    nc.tensor.matmul(ots[(p, 1)][:], v_aug[:, kb, :], e[:, 1, :],
                                     start=st_, stop=sp_)

                def do_evacs(p, qq_):
                    # one [128,512] f32 copy per head frees the PSUM bank in a
                    # single ACT op; recip on DVE; bc matmul + mul issued on
                    # later ticks so nothing waits on this chain.
                    recip2 = tmpB.tile([1, 2, 512], F32, tag=f"recip{p}", bufs=2)
                    recip2b = tmpB.tile([1, 2, 512], BF16, tag=f"recipb{p}", bufs=2)
                    for i in range(2):
                        ou = tmpB.tile([128, 512], F32, tag="ou", bufs=4)
                        nc.scalar.copy(ou[:], ots[(p, i)][:, :])
                        nc.vector.reciprocal_approx_fast(recip2[0:1, i, :], ou[0:1, :])
                        pending_bc[p].append((qq_, i, ou, recip2b))
                    nc.scalar.copy(recip2b[:], recip2[:])

                def do_bc(p):
                    while pending_bc[p]:
                        qq_, i, ou, recip2b = pending_bc[p].pop(0)
                        bc = psB.tile([64, 512], F32, name=f"bc{p}{i}",
                                      tag=f"ot{2 * p + i}")
                        nc.tensor.matmul(
                            bc[:], bco_sb[:], recip2b[0:1, i, :],
                            start=True, stop=True,
                        )
                        pending_mul[p].append((qq_, i, ou, bc))

                def flush_mul(p):
                    while pending_mul[p]:
                        qq_, i, ou, bc = pending_mul[p].pop(0)
                        h = 2 * p + i
                        jb, rr = divmod(h, 2)
                        nc.vector.tensor_mul(
                            oT[jb][rr * 64:rr * 64 + 64, bass.ts(qq_, 512)],
                            ou[64:128, :], bc[0:64, :],
                        )

                NB = NS * KC  # 64 global key-block steps per pair
                for t in range(NB + 1):
                    for p in (0, 1):
                        g = t - p
                        if g < 0 or g >= NB:
                            continue
                        qq, kb = divmod(g, KC)
                        qsl = bass.ts(qq, 512)
                        if kb == 0 and prev[p] is not None:
                            # finish the previous span: last av into the OLD
                            # accumulators, then evacuate them in one ACT op
                            # per head (+ recip on DVE).
                            av(p, prev[p])
                            prev[p] = None
                            do_evacs(p, qq - 1)
                        if kb == 1:
                            if qq > 0:
                                do_bc(p)       # PE, recips are done by now
                            if qq > 0:
                                flush_mul(p)   # DVE, before this tick's exp
                            for i in range(2):
                                ots[(p, i)] = psB.tile(
                                    [128, 512], F32,
                                    name=f"ot{2 * p + i}", tag=f"ot{2 * p + i}",
                                    bufs=1,
                                )
                        st_ps = psB.tile([128, 2, 512], F32, tag="st", bufs=2)
                        nc.tensor.matmul(
                            st_ps[:, 0, :],
                            kTr[0:64, kb * 128:(kb + 1) * 128],
                            qTr[p][0:64, qsl], start=True, stop=True,
                        )
                        nc.tensor.matmul(
                            st_ps[:, 1, :],
                            kTr[64:128, kb * 128:(kb + 1) * 128],
                            qTr[p][64:128, qsl], start=True, stop=True,
                        )
                        e_sb = tmpB.tile([128, 2, 512], BF16, tag="e", bufs=4)
                        on_act = (kb % 2 == 0 and kb != 14) or kb == 15
                        if on_act:
                            nc.scalar.activation(e_sb[:], st_ps[:], AF.Exp,
                                                 scale=SCALE)
                        else:
                            nc.vector.tensor_scalar(
                                e_sb.bitcast(INT16)[:], st_ps[:],
                                C0S, C1S,
                                mybir.AluOpType.mult, mybir.AluOpType.add,
                            )
                        if prev[p] is not None:
                            av(p, prev[p])
                        prev[p] = (qq, kb, e_sb)
                for p in (0, 1):
                    av(p, prev[p])
                    do_evacs(p, NS - 1)
                    do_bc(p)
                    flush_mul(p)

            # ---------------- stage C tail: out = oT.T @ wo, bf16 store.
            # One batched row-DMA per srow (4 copies land in one ob tile) —
            # per-DMA post cost on the Sync engine is ~0.6us, so 16 posts
            # instead of 64 matters.
            with (
                tc.tile_pool(name="psC", bufs=1, space="PSUM") as psC,
                tc.tile_pool(name="outp", bufs=3) as outp,
            ):
                for srow in range(S // 128):
                    ob = outp.tile([128, D], BF16, tag="ob")
                    for nn in range(NS):
                        o_ps = psC.tile([128, 512], F32, tag="oc", bufs=4)
                        nc.tensor.matmul(
                            o_ps[:], oT[0][:, srow * 128:(srow + 1) * 128],
                            wo_sb[:, 0, bass.ts(nn, 512)], start=True, stop=False,
                        )
                        nc.tensor.matmul(
                            o_ps[:], oT[1][:, srow * 128:(srow + 1) * 128],
                            wo_sb[:, 1, bass.ts(nn, 512)], start=False, stop=True,
                        )
                        if nn % 2 == 0:
                            nc.vector.tensor_copy(ob[:, bass.ts(nn, 512)], o_ps[:])
                        else:
                            nc.scalar.copy(ob[:, bass.ts(nn, 512)], o_ps[:])
                    nc.sync.dma_start(
                        out_d[srow * 128:(srow + 1) * 128, :], ob[:]
                    )
    nc.compile()
    return nc


_NC_CACHE = None


def _get_program():
    global _NC_CACHE
    if _NC_CACHE is None:
        _NC_CACHE = _build_program()
    return _NC_CACHE


def _host_constants():
    inv_freq = 1.0 / (ROPE_BASE ** (np.arange(0, HD, 2, dtype=np.float32) / HD))
    t = np.arange(S, dtype=np.float32)
    freqs = np.outer(t, inv_freq)
    emb = np.concatenate([freqs, freqs], -1)          # [s, 64]
    cosT = np.cos(emb).T.astype(np.float32)           # [64, s]
    sinT = np.sin(emb).T.astype(np.float32)
    cos2 = np.ascontiguousarray(np.concatenate([cosT, cosT], 0)).astype(NPBF16)
    sin2 = np.ascontiguousarray(np.concatenate([sinT, sinT], 0)).astype(NPBF16)

    R = np.zeros((HD, HD), np.float32)
    for i in range(32):
        R[i, i + 32] = -1.0
        R[i + 32, i] = 1.0
    RT = R.T
    rot_q = np.zeros((128, 128), np.float32)
    rot_q[0:64, 0:64] = RT
    rot_q[64:128, 64:128] = RT
    rot_k = np.zeros((128, 64), np.float32)
    rot_k[0:64, 0:64] = RT
    id64 = np.zeros((128, 64), np.float32)
    id64[64:128, :] = np.eye(64, dtype=np.float32)
    ones_col = np.ones((128, KC), np.float32)
    return (cos2, sin2, rot_q.astype(NPBF16), rot_k.astype(NPBF16),
            id64.astype(NPBF16), ones_col.astype(NPBF16))


def _in_maps(x, wq, wk, wv, wo):
    xT = np.ascontiguousarray(x.reshape(S, D).T.astype(NPBF16))
    cos2, sin2, rot_q, rot_k, id64, ones_col = _host_constants()
    maps = []
    for c in range(NCORES):
        wq_c = wq[:, c * QW:(c + 1) * QW].astype(NPBF16)
        wq_p = np.ascontiguousarray(wq_c.reshape(KC, 128, QW).transpose(1, 0, 2))
        wkv_c = np.concatenate(
            [wk[:, c * HD:(c + 1) * HD], wv[:, c * HD:(c + 1) * HD]], 1
        ).astype(NPBF16)
        wkv_p = np.ascontiguousarray(wkv_c.reshape(KC, 128, 128).transpose(1, 0, 2))
        wo_c = wo[c * QW:(c + 1) * QW, :].astype(NPBF16)
        wo_p = np.ascontiguousarray(wo_c.reshape(2, 128, D).transpose(1, 0, 2))
        maps.append({
            "xt": xT, "wq_p": wq_p, "wkv_p": wkv_p, "wo_p": wo_p,
            "cos2": cos2, "sin2": sin2, "rot_q": rot_q, "rot_k": rot_k,
            "id64": id64, "ones_col": ones_col,
            "bc_ones": np.ones((1, 64), np.float32).astype(NPBF16),
        })
    return maps


def _run(in_maps, trace=False):
    nc = _get_program()
    return run_bass_kernel_spmd(nc, in_maps, core_ids=list(range(NCORES)), trace=trace)


def _gather(res):
    acc = res.results[0]["out"].astype(np.float64)
    for c in range(1, NCORES):
        acc += res.results[c]["out"].astype(np.float64)
    return acc.astype(np.float32).reshape(1, S, D)


def kernel(x, wq, wk, wv, wo):
    x, wq, wk, wv, wo = (np.asarray(a, dtype=np.float32) for a in (x, wq, wk, wv, wo))
    res = _run(_in_maps(x, wq, wk, wv, wo), trace=False)
    return _gather(res)


def run_traced(x, wq, wk, wv, wo):
    """Like kernel() but with NTFF profiling; returns (out, BassKernelResults)."""
    x, wq, wk, wv, wo = (np.asarray(a, dtype=np.float32) for a in (x, wq, wk, wv, wo))
    res = _run(_in_maps(x, wq, wk, wv, wo), trace=True)
    return _gather(res), res
